# revision 6
# baseline (speedup 1.0000x reference)
"""Multi-head attention (b=4, n=2048, dim=768, 12 heads) on 8 TRN2 NeuronCores.

Sharding: core c handles batch c//2 and head-group c%2 (6 of 12 heads).  Each
core computes its heads' contribution projected through its slice of Wo and
returns a partial [2048, 768] bf16 output; the host sums core pairs in f32 and
adds the bias.  No on-device collectives needed.

Per-core kernel (TensorE-facing data in bf16, accumulation in f32):
  P1: KT/QT = W^T x^T feature-major (x^T supplied pre-transposed by the host),
      V token-major in 128-wide head blocks whose column 64 is constant 1.
      Inputs land over four parallel DMA queues (sync/vector/scalar/gpsimd).
  P2: scores computed TRANSPOSED: ST[j,i] = K Q^T on PE (K=64 contraction),
      exp on ACT with scale 1/8 (softmax max-subtraction is unnecessary at
      these logit magnitudes), then OP[., i] += V'^T exp(ST) accumulated over
      j in PSUM.  The ones column of V' makes row 64 of OP the softmax
      denominator l[i] for free.  Unit order is (i-block, head, j-chunk) with
      4 j-chunks per exp tile; the attV matmuls trail the score matmuls by
      two packs so the PE never waits on ACT.
  P3: per 128 tokens: PE-transpose OP (bf16) to token-major, multiply by 1/l
      (per-partition scalar), PE-transpose back, project through Wo with
      PSUM accumulation.  P3 jobs for i-block k are interleaved between the
      P2 packs of i-block k+1, so there is no serial tail phase and output
      DMAs (bf16) spread across the whole run.
"""
import os
import sys
import types
from collections import deque
import numpy as np
import ml_dtypes

B, N, DIM = 4, 2048, 768
HEADS, DH = 12, 64
HPC = 6                # heads per core
FPC = HPC * DH         # 384 features per core
NCORES = 8
KC = DIM // 128        # 6 contraction chunks
FT = FPC // 128        # 3 feature tiles per core
NT = N // 128          # 16 token chunks of 128
IBS = 256              # i-block size
IB = N // IBS          # 8 i-blocks
JPP = 4                # j-chunks per pack (exp tile = [128, JPP*IBS])
PPH = NT // JPP        # 4 packs per (ib, h)
AVLAG = 2              # attV trails scores by this many packs
BF16 = ml_dtypes.bfloat16

_cache = {}
last_exec_time_ns = None


def _install_ntff_hook():
    try:
        import antenv.axon_hooks  # noqa: F401
        return
    except ImportError:
        pass
    from trn_agent_boot.trn_boot import _ntff_profile_via_ctypes
    hook = _ntff_profile_via_ctypes('/opt/axon/libaxon_pjrt.so')
    mod = types.ModuleType('antenv.axon_hooks')
    mod.get_axon_ntff_profile_hook = lambda: hook
    import antenv
    sys.modules['antenv.axon_hooks'] = mod
    antenv.axon_hooks = mod


def _build_nc():
    from contextlib import ExitStack
    from concourse import bacc
    import concourse.mybir as mybir
    from concourse.tile import TileContext
    from concourse.masks import make_identity
    from concourse.bass import broadcast_tensor_aps

    dt = mybir.dt
    EXP = mybir.ActivationFunctionType.Exp

    nc = bacc.Bacc("TRN2", target_bir_lowering=False, debug=False,
                   num_devices=NCORES)
    xT = nc.dram_tensor("xT", [DIM, N], dt.bfloat16, kind="ExternalInput").ap()
    wq = nc.dram_tensor("wq", [DIM, FPC], dt.bfloat16, kind="ExternalInput").ap()
    wk = nc.dram_tensor("wk", [DIM, FPC], dt.bfloat16, kind="ExternalInput").ap()
    wv = nc.dram_tensor("wv", [DIM, FPC], dt.bfloat16, kind="ExternalInput").ap()
    wo = nc.dram_tensor("wo", [FPC, DIM], dt.bfloat16, kind="ExternalInput").ap()
    out = nc.dram_tensor("out", [N, DIM], dt.bfloat16, kind="ExternalOutput").ap()

    with TileContext(nc) as tc, ExitStack() as ctx:
        const = ctx.enter_context(tc.tile_pool(name="const", bufs=1))
        id_bf = const.tile([128, 128], dt.bfloat16, tag="idb")
        make_identity(nc, id_bf)

        inp = ctx.enter_context(tc.tile_pool(name="inp", bufs=1))
        xts2 = [[inp.tile([128, N // 2], dt.bfloat16, tag=f"xt{k}_{hf}",
                          name=f"xt{k}_{hf}") for hf in range(2)]
                for k in range(KC)]
        wqs = [inp.tile([128, FPC], dt.bfloat16, tag=f"wq{k}", name=f"wq{k}")
               for k in range(KC)]
        wks = [inp.tile([128, FPC], dt.bfloat16, tag=f"wk{k}", name=f"wk{k}")
               for k in range(KC)]
        wvs = [inp.tile([128, FPC], dt.bfloat16, tag=f"wv{k}", name=f"wv{k}")
               for k in range(KC)]
        wos = [inp.tile([128, DIM], dt.bfloat16, tag=f"wo{f}", name=f"wo{f}")
               for f in range(FT)]
        # Two parallel DMA queues; ordering keeps the V projection fed from
        # ~1us (wv, x half 0) and both x halves landed by ~6us.
        for k in range(KC):
            nc.sync.dma_start(out=xts2[k][0][:],
                              in_=xT[k * 128:(k + 1) * 128, 0:N // 2])
            nc.scalar.dma_start(out=wvs[k][:], in_=wv[k * 128:(k + 1) * 128, :])
        for k in range(KC):
            nc.scalar.dma_start(out=xts2[k][1][:],
                               in_=xT[k * 128:(k + 1) * 128, N // 2:N])
        for k in range(KC):
            nc.scalar.dma_start(out=wks[k][:], in_=wk[k * 128:(k + 1) * 128, :])
        for k in range(KC):
            nc.scalar.dma_start(out=wqs[k][:], in_=wq[k * 128:(k + 1) * 128, :])
        for f in range(FT):
            nc.scalar.dma_start(out=wos[f][:], in_=wo[f * 128:(f + 1) * 128, :])

        kqv = ctx.enter_context(tc.tile_pool(name="kqv", bufs=1))
        KT = [kqv.tile([128, N], dt.bfloat16, tag=f"kt{f}", name=f"kt{f}")
              for f in range(FT)]
        QT = [kqv.tile([128, N], dt.bfloat16, tag=f"qt{f}", name=f"qt{f}")
              for f in range(FT)]
        VP = [kqv.tile([128, HPC * 128], dt.bfloat16, tag=f"vp{t}", name=f"vp{t}")
              for t in range(NT)]
        opsb = ctx.enter_context(tc.tile_pool(name="opsb", bufs=1))
        OPS = [[opsb.tile([65, IBS], dt.bfloat16, tag=f"op{h}_{ib}",
                          name=f"op{h}_{ib}") for ib in range(IB)]
               for h in range(HPC)]

        # ---- P1: projections ----
        for t in range(NT):
            nc.vector.memset(
                VP[t].rearrange("p (h c) -> p h c", c=128)[:, :, 64:65], 1.0)
        with tc.tile_pool(name="p1ps", bufs=8, space="PSUM") as p1:
            for t in range(NT):
                ps = p1.tile([128, FPC], dt.float32, tag="p1", name=f"vps{t}")
                for k in range(KC):
                    nc.tensor.matmul(
                        ps[:],
                        lhsT=xts2[k][t // 8][:, (t % 8) * 128:(t % 8 + 1) * 128],
                        rhs=wvs[k][:], start=(k == 0), stop=(k == KC - 1))
                nc.vector.tensor_copy(
                    VP[t].rearrange("p (h c) -> p h c", c=128)[:, :, 0:64],
                    ps.rearrange("p (h c) -> p h c", c=64))
            for W, DST in ((wks, KT), (wqs, QT)):
                for f in range(FT):
                    for q in range(N // 512):
                        ps = p1.tile([128, 512], dt.float32, tag="p1",
                                     name=f"kqps{f}_{q}")
                        for k in range(KC):
                            nc.tensor.matmul(
                                ps[:], lhsT=W[k][:, f * 128:(f + 1) * 128],
                                rhs=xts2[k][q // 2][:, (q % 2) * 512:
                                                    (q % 2 + 1) * 512],
                                start=(k == 0), stop=(k == KC - 1))
                        nc.vector.tensor_copy(DST[f][:, q * 512:(q + 1) * 512],
                                              ps[:])

        # ---- P2 + P3 fused ----
        packs = [(ib, h, g) for ib in range(IB) for h in range(HPC)
                 for g in range(PPH)]
        NP = len(packs)

        with tc.tile_pool(name="p2st", bufs=2, space="PSUM") as p2st, \
                tc.tile_pool(name="p2op", bufs=2, space="PSUM") as p2op, \
                tc.tile_pool(name="p3ps", bufs=2, space="PSUM") as p3ps, \
                tc.tile_pool(name="expp", bufs=3) as expp, \
                tc.tile_pool(name="otokp", bufs=2) as otokp, \
                tc.tile_pool(name="otnp", bufs=2) as otnp, \
                tc.tile_pool(name="linvp", bufs=2) as linvp, \
                tc.tile_pool(name="outst", bufs=2) as outst:
            ops = {}
            exs = {}

            def sc_emit(p):
                ib, h, g = packs[p]
                ktf, qtf, r0 = KT[h // 2], QT[h // 2], (h % 2) * 64
                st = p2st.tile([128, JPP * IBS], dt.float32, tag="st",
                               name=f"st{p}")
                for u in range(JPP):
                    jc = g * JPP + u
                    nc.tensor.matmul(
                        st[:, u * IBS:(u + 1) * IBS],
                        lhsT=ktf[r0:r0 + 64, jc * 128:(jc + 1) * 128],
                        rhs=qtf[r0:r0 + 64, ib * IBS:(ib + 1) * IBS],
                        start=True, stop=True)
                ex = expp.tile([128, JPP * IBS], dt.bfloat16, tag="ex",
                               name=f"ex{p}")
                nc.scalar.activation(ex[:], st[:], EXP, scale=0.125)
                exs[p] = ex

            def av_emit(p):
                ib, h, g = packs[p]
                ex = exs.pop(p)
                if g == 0:
                    ops[(h, ib)] = p2op.tile([128, IBS], dt.float32,
                                             tag="op", name=f"opp{h}_{ib}")
                op = ops[(h, ib)]
                for u in range(JPP):
                    jc = g * JPP + u
                    nc.tensor.matmul(
                        op[:], lhsT=VP[jc][:, h * 128:(h + 1) * 128],
                        rhs=ex[:, u * IBS:(u + 1) * IBS],
                        start=(jc == 0), stop=(jc == NT - 1))
                if g == PPH - 1:
                    op = ops.pop((h, ib))
                    nc.vector.tensor_copy(OPS[h][ib][:], op[0:65, :])

            otoks = {}
            otns = {}

            def tr_job(isub):
                ib, col = isub // (IBS // 128), (isub % (IBS // 128)) * 128
                # 66-wide per-head slots keep bf16 PSUM writes 4B-aligned
                trp = p3ps.tile([128, HPC * 66], dt.bfloat16, tag="p3",
                                name=f"trp{isub}")
                for h in range(HPC):
                    nc.tensor.transpose(trp[:, h * 66:h * 66 + 65],
                                        OPS[h][ib][0:65, col:col + 128],
                                        id_bf[0:65, 0:65])
                trv = trp.rearrange("p (h c) -> p h c", c=66)
                linv6 = linvp.tile([128, HPC, 1], dt.float32, tag="l6",
                                   name=f"l6{isub}")
                nc.vector.reciprocal(linv6[:], trv[:, :, 64:65])
                otok = otokp.tile([128, FPC], dt.bfloat16, tag="otok",
                                  name=f"otok{isub}")
                a, b = broadcast_tensor_aps(trv[:, :, 0:64], linv6[:])
                nc.vector.tensor_mul(
                    otok.rearrange("p (h c) -> p h c", c=64), a, b)
                otoks[isub] = otok

            def tb_job(isub):
                otok = otoks.pop(isub)
                tbp = p3ps.tile([128, FPC], dt.bfloat16, tag="p3",
                                name=f"tbp{isub}")
                for f in range(FT):
                    nc.tensor.transpose(tbp[:, f * 128:(f + 1) * 128],
                                        otok[:, f * 128:(f + 1) * 128],
                                        id_bf[:])
                otn = otnp.tile([128, FPC], dt.bfloat16, tag="otn",
                                name=f"otn{isub}")
                nc.vector.tensor_copy(otn[:], tbp[:])
                otns[isub] = otn

            def proj_job(isub):
                otn = otns.pop(isub)
                ob = outst.tile([128, DIM], dt.bfloat16, tag="ob",
                                name=f"ob{isub}")
                for half in range(2):
                    pp = p3ps.tile([128, DIM // 2], dt.float32, tag="p3",
                                   name=f"pp{isub}_{half}")
                    for f in range(FT):
                        nc.tensor.matmul(
                            pp[:], lhsT=otn[:, f * 128:(f + 1) * 128],
                            rhs=wos[f][:, half * 384:(half + 1) * 384],
                            start=(f == 0), stop=(f == FT - 1))
                    nc.vector.tensor_copy(ob[:, half * 384:(half + 1) * 384],
                                          pp[:])
                nc.sync.dma_start(out=out[isub * 128:(isub + 1) * 128, :],
                                  in_=ob[:])

            p3jobs = deque()

            def queue_ib(ib):
                subs = range(ib * (IBS // 128), (ib + 1) * (IBS // 128))
                for fn in (tr_job, tb_job, proj_job):
                    for s in subs:
                        p3jobs.append((fn, s))

            for p in range(NP):
                if p >= AVLAG:
                    q = p - AVLAG
                    av_emit(q)
                    if q % (HPC * PPH) == HPC * PPH - 1:
                        queue_ib(q // (HPC * PPH))
                sc_emit(p)
                if p3jobs and p % 3 == 2:
                    fn, s = p3jobs.popleft()
                    fn(s)
            for q in range(NP - AVLAG, NP):
                av_emit(q)
            queue_ib(IB - 1)
            while p3jobs:
                fn, s = p3jobs.popleft()
                fn(s)

    nc.finalize()
    return nc


def _get_nc():
    if "nc" not in _cache:
        _cache["nc"] = _build_nc()
    return _cache["nc"]


def kernel(x, Wq, Wk, Wv, Wo, bo):
    global last_exec_time_ns
    x = np.asarray(x, dtype=np.float32)
    Wq = np.asarray(Wq, dtype=np.float32)
    Wk = np.asarray(Wk, dtype=np.float32)
    Wv = np.asarray(Wv, dtype=np.float32)
    Wo = np.asarray(Wo, dtype=np.float32)
    bo = np.asarray(bo, dtype=np.float32)

    trace = bool(os.environ.get("BASS_KERNEL_TRACE"))
    if trace:
        _install_ntff_hook()
        import concourse.bass_utils as bass_utils
        bass_utils.upload_artifacts = lambda tmpdir: tmpdir

    nc = _get_nc()
    in_maps = []
    for c in range(NCORES):
        bi, hg = divmod(c, 2)
        s = slice(hg * FPC, (hg + 1) * FPC)
        in_maps.append({
            "xT": np.ascontiguousarray(x[bi].T).astype(BF16),
            "wq": np.ascontiguousarray(Wq[:, s]).astype(BF16),
            "wk": np.ascontiguousarray(Wk[:, s]).astype(BF16),
            "wv": np.ascontiguousarray(Wv[:, s]).astype(BF16),
            "wo": np.ascontiguousarray(Wo[s, :]).astype(BF16),
        })

    from concourse.bass_utils import run_bass_kernel_spmd
    res = run_bass_kernel_spmd(nc, in_maps, list(range(NCORES)), trace=trace)
    last_exec_time_ns = res.exec_time_ns

    parts = [res.results[c]["out"] for c in range(NCORES)]
    full = np.empty((B, N, DIM), np.float32)
    for bi in range(B):
        full[bi] = (parts[2 * bi].astype(np.float32)
                    + parts[2 * bi + 1].astype(np.float32) + bo[None, :])
    return full


# revision 7
# speedup vs baseline: 1.0700x; 1.0700x over previous
"""Multi-head attention (b=4, n=2048, dim=768, 12 heads) on 8 TRN2 NeuronCores.

Sharding: core c handles batch c//2 and head-group c%2 (6 of 12 heads).  Each
core computes its heads' contribution projected through its slice of Wo and
returns a partial [2048, 768] bf16 output; the host sums core pairs in f32 and
adds the bias.  No on-device collectives needed.

Per-core kernel (TensorE-facing data in bf16, accumulation in f32):
  P1: KT/QT = W^T x^T feature-major (x^T supplied pre-transposed by the host),
      V token-major in 128-wide head blocks whose column 64 is constant 1.
      Inputs land over four parallel DMA queues (sync/vector/scalar/gpsimd).
  P2: scores computed TRANSPOSED: ST[j,i] = K Q^T on PE (K=64 contraction),
      exp on ACT with scale 1/8 (softmax max-subtraction is unnecessary at
      these logit magnitudes), then OP[., i] += V'^T exp(ST) accumulated over
      j in PSUM.  The ones column of V' makes row 64 of OP the softmax
      denominator l[i] for free.  Unit order is (i-block, head, j-chunk) with
      4 j-chunks per exp tile; the attV matmuls trail the score matmuls by
      two packs so the PE never waits on ACT.
  P3: per 128 tokens: PE-transpose OP (bf16) to token-major, multiply by 1/l
      (per-partition scalar), PE-transpose back, project through Wo with
      PSUM accumulation.  P3 jobs for i-block k are interleaved between the
      P2 packs of i-block k+1, so there is no serial tail phase and output
      DMAs (bf16) spread across the whole run.
"""
import os
import sys
import types
from collections import deque
import numpy as np
import ml_dtypes

B, N, DIM = 4, 2048, 768
HEADS, DH = 12, 64
HPC = 6                # heads per core
FPC = HPC * DH         # 384 features per core
NCORES = 8
KC = DIM // 128        # 6 contraction chunks
FT = FPC // 128        # 3 feature tiles per core
NT = N // 128          # 16 token chunks of 128
IBS = 256              # i-block size
IB = N // IBS          # 8 i-blocks
JPP = 4                # j-chunks per pack (exp tile = [128, JPP*IBS])
PPH = NT // JPP        # 4 packs per (ib, h)
AVLAG = 2              # attV trails scores by this many packs
BF16 = ml_dtypes.bfloat16

_cache = {}
last_exec_time_ns = None


def _install_ntff_hook():
    try:
        import antenv.axon_hooks  # noqa: F401
        return
    except ImportError:
        pass
    from trn_agent_boot.trn_boot import _ntff_profile_via_ctypes
    hook = _ntff_profile_via_ctypes('/opt/axon/libaxon_pjrt.so')
    mod = types.ModuleType('antenv.axon_hooks')
    mod.get_axon_ntff_profile_hook = lambda: hook
    import antenv
    sys.modules['antenv.axon_hooks'] = mod
    antenv.axon_hooks = mod


def _build_nc():
    from contextlib import ExitStack
    from concourse import bacc
    import concourse.mybir as mybir
    from concourse.tile import TileContext
    from concourse.masks import make_identity
    from concourse.bass import broadcast_tensor_aps

    dt = mybir.dt
    EXP = mybir.ActivationFunctionType.Exp

    nc = bacc.Bacc("TRN2", target_bir_lowering=False, debug=False,
                   num_devices=NCORES)
    xT = nc.dram_tensor("xT", [DIM, N], dt.bfloat16, kind="ExternalInput").ap()
    wq = nc.dram_tensor("wq", [DIM, FPC], dt.bfloat16, kind="ExternalInput").ap()
    wk = nc.dram_tensor("wk", [DIM, FPC], dt.bfloat16, kind="ExternalInput").ap()
    wv = nc.dram_tensor("wv", [DIM, FPC], dt.bfloat16, kind="ExternalInput").ap()
    wo = nc.dram_tensor("wo", [FPC, DIM], dt.bfloat16, kind="ExternalInput").ap()
    out = nc.dram_tensor("out", [N, DIM], dt.bfloat16, kind="ExternalOutput").ap()

    with TileContext(nc) as tc, ExitStack() as ctx:
        const = ctx.enter_context(tc.tile_pool(name="const", bufs=1))
        id_bf = const.tile([128, 128], dt.bfloat16, tag="idb")
        make_identity(nc, id_bf)

        inp = ctx.enter_context(tc.tile_pool(name="inp", bufs=1))
        xts2 = [[inp.tile([128, N // 2], dt.bfloat16, tag=f"xt{k}_{hf}",
                          name=f"xt{k}_{hf}") for hf in range(2)]
                for k in range(KC)]
        wqs = [inp.tile([128, FPC], dt.bfloat16, tag=f"wq{k}", name=f"wq{k}")
               for k in range(KC)]
        wks = [inp.tile([128, FPC], dt.bfloat16, tag=f"wk{k}", name=f"wk{k}")
               for k in range(KC)]
        wvs = [inp.tile([128, FPC], dt.bfloat16, tag=f"wv{k}", name=f"wv{k}")
               for k in range(KC)]
        wos = [inp.tile([128, DIM], dt.bfloat16, tag=f"wo{f}", name=f"wo{f}")
               for f in range(FT)]
        # Two parallel DMA queues; ordering keeps the V projection fed from
        # ~1us (wv, x half 0) and both x halves landed by ~6us.
        for k in range(KC):
            nc.sync.dma_start(out=xts2[k][0][:],
                              in_=xT[k * 128:(k + 1) * 128, 0:N // 2])
            nc.scalar.dma_start(out=wvs[k][:], in_=wv[k * 128:(k + 1) * 128, :])
        for k in range(KC):
            nc.scalar.dma_start(out=xts2[k][1][:],
                               in_=xT[k * 128:(k + 1) * 128, N // 2:N])
        for k in range(KC):
            nc.scalar.dma_start(out=wks[k][:], in_=wk[k * 128:(k + 1) * 128, :])
        for k in range(KC):
            nc.scalar.dma_start(out=wqs[k][:], in_=wq[k * 128:(k + 1) * 128, :])
        for f in range(FT):
            nc.scalar.dma_start(out=wos[f][:], in_=wo[f * 128:(f + 1) * 128, :])

        kqv = ctx.enter_context(tc.tile_pool(name="kqv", bufs=1))
        KT = [kqv.tile([128, N], dt.bfloat16, tag=f"kt{f}", name=f"kt{f}")
              for f in range(FT)]
        QT = [kqv.tile([128, N], dt.bfloat16, tag=f"qt{f}", name=f"qt{f}")
              for f in range(FT)]
        VP = [kqv.tile([128, HPC * 128], dt.bfloat16, tag=f"vp{t}", name=f"vp{t}")
              for t in range(NT)]
        opsb = ctx.enter_context(tc.tile_pool(name="opsb", bufs=1))
        OPS = [[opsb.tile([65, IBS], dt.bfloat16, tag=f"op{h}_{ib}",
                          name=f"op{h}_{ib}") for ib in range(IB)]
               for h in range(HPC)]

        # ---- P1: projections ----
        for t in range(NT):
            nc.vector.memset(
                VP[t].rearrange("p (h c) -> p h c", c=128)[:, :, 64:65], 1.0)
        with tc.tile_pool(name="p1ps", bufs=8, space="PSUM") as p1:
            # V projection k-major over 8 tiles at a time so the PE stream
            # never blocks behind a matmul whose x chunk has not landed yet.
            for half in range(2):
                pss = [p1.tile([128, FPC], dt.float32, tag="p1",
                               name=f"vps{half}_{i}") for i in range(8)]
                for k in range(KC):
                    for i in range(8):
                        nc.tensor.matmul(
                            pss[i][:],
                            lhsT=xts2[k][half][:, i * 128:(i + 1) * 128],
                            rhs=wvs[k][:], start=(k == 0), stop=(k == KC - 1))
                for i in range(8):
                    nc.vector.tensor_copy(
                        VP[half * 8 + i].rearrange(
                            "p (h c) -> p h c", c=128)[:, :, 0:64],
                        pss[i].rearrange("p (h c) -> p h c", c=64))
            # K and Q interleaved per feature tile so the first P2 packs
            # (which need KT[0] and QT[0]) are unblocked as early as possible.
            for f in range(FT):
                for W, DST in ((wks, KT), (wqs, QT)):
                    for q in range(N // 512):
                        ps = p1.tile([128, 512], dt.float32, tag="p1",
                                     name=f"kqps{f}_{q}")
                        for k in range(KC):
                            nc.tensor.matmul(
                                ps[:], lhsT=W[k][:, f * 128:(f + 1) * 128],
                                rhs=xts2[k][q // 2][:, (q % 2) * 512:
                                                    (q % 2 + 1) * 512],
                                start=(k == 0), stop=(k == KC - 1))
                        nc.vector.tensor_copy(DST[f][:, q * 512:(q + 1) * 512],
                                              ps[:])

        # ---- P2 + P3 fused ----
        packs = [(ib, h, g) for ib in range(IB) for h in range(HPC)
                 for g in range(PPH)]
        NP = len(packs)

        with tc.tile_pool(name="p2st", bufs=2, space="PSUM") as p2st, \
                tc.tile_pool(name="p2op", bufs=2, space="PSUM") as p2op, \
                tc.tile_pool(name="p3ps", bufs=2, space="PSUM") as p3ps, \
                tc.tile_pool(name="expp", bufs=3) as expp, \
                tc.tile_pool(name="otokp", bufs=2) as otokp, \
                tc.tile_pool(name="otnp", bufs=2) as otnp, \
                tc.tile_pool(name="linvp", bufs=2) as linvp, \
                tc.tile_pool(name="outst", bufs=2) as outst:
            ops = {}
            exs = {}

            def sc_emit(p):
                ib, h, g = packs[p]
                ktf, qtf, r0 = KT[h // 2], QT[h // 2], (h % 2) * 64
                st = p2st.tile([128, JPP * IBS], dt.float32, tag="st",
                               name=f"st{p}")
                for u in range(JPP):
                    jc = g * JPP + u
                    nc.tensor.matmul(
                        st[:, u * IBS:(u + 1) * IBS],
                        lhsT=ktf[r0:r0 + 64, jc * 128:(jc + 1) * 128],
                        rhs=qtf[r0:r0 + 64, ib * IBS:(ib + 1) * IBS],
                        start=True, stop=True)
                ex = expp.tile([128, JPP * IBS], dt.bfloat16, tag="ex",
                               name=f"ex{p}")
                nc.scalar.activation(ex[:], st[:], EXP, scale=0.125)
                exs[p] = ex

            def av_emit(p):
                ib, h, g = packs[p]
                ex = exs.pop(p)
                if g == 0:
                    ops[(h, ib)] = p2op.tile([128, IBS], dt.float32,
                                             tag="op", name=f"opp{h}_{ib}")
                op = ops[(h, ib)]
                for u in range(JPP):
                    jc = g * JPP + u
                    nc.tensor.matmul(
                        op[:], lhsT=VP[jc][:, h * 128:(h + 1) * 128],
                        rhs=ex[:, u * IBS:(u + 1) * IBS],
                        start=(jc == 0), stop=(jc == NT - 1))
                if g == PPH - 1:
                    op = ops.pop((h, ib))
                    nc.vector.tensor_copy(OPS[h][ib][:], op[0:65, :])

            otoks = {}
            otns = {}

            def tr_job(isub):
                ib, col = isub // (IBS // 128), (isub % (IBS // 128)) * 128
                # 66-wide per-head slots keep bf16 PSUM writes 4B-aligned
                trp = p3ps.tile([128, HPC * 66], dt.bfloat16, tag="p3",
                                name=f"trp{isub}")
                for h in range(HPC):
                    nc.tensor.transpose(trp[:, h * 66:h * 66 + 65],
                                        OPS[h][ib][0:65, col:col + 128],
                                        id_bf[0:65, 0:65])
                trv = trp.rearrange("p (h c) -> p h c", c=66)
                linv6 = linvp.tile([128, HPC, 1], dt.float32, tag="l6",
                                   name=f"l6{isub}")
                nc.vector.reciprocal(linv6[:], trv[:, :, 64:65])
                otok = otokp.tile([128, FPC], dt.bfloat16, tag="otok",
                                  name=f"otok{isub}")
                a, b = broadcast_tensor_aps(trv[:, :, 0:64], linv6[:])
                nc.vector.tensor_mul(
                    otok.rearrange("p (h c) -> p h c", c=64), a, b)
                otoks[isub] = otok

            def tb_job(isub):
                otok = otoks.pop(isub)
                tbp = p3ps.tile([128, FPC], dt.bfloat16, tag="p3",
                                name=f"tbp{isub}")
                for f in range(FT):
                    nc.tensor.transpose(tbp[:, f * 128:(f + 1) * 128],
                                        otok[:, f * 128:(f + 1) * 128],
                                        id_bf[:])
                otn = otnp.tile([128, FPC], dt.bfloat16, tag="otn",
                                name=f"otn{isub}")
                nc.vector.tensor_copy(otn[:], tbp[:])
                otns[isub] = otn

            def proj_job(isub):
                otn = otns.pop(isub)
                ob = outst.tile([128, DIM], dt.bfloat16, tag="ob",
                                name=f"ob{isub}")
                for half in range(2):
                    pp = p3ps.tile([128, DIM // 2], dt.float32, tag="p3",
                                   name=f"pp{isub}_{half}")
                    for f in range(FT):
                        nc.tensor.matmul(
                            pp[:], lhsT=otn[:, f * 128:(f + 1) * 128],
                            rhs=wos[f][:, half * 384:(half + 1) * 384],
                            start=(f == 0), stop=(f == FT - 1))
                    nc.vector.tensor_copy(ob[:, half * 384:(half + 1) * 384],
                                          pp[:])
                nc.sync.dma_start(out=out[isub * 128:(isub + 1) * 128, :],
                                  in_=ob[:])

            p3jobs = deque()

            def queue_ib(ib):
                subs = range(ib * (IBS // 128), (ib + 1) * (IBS // 128))
                for fn in (tr_job, tb_job, proj_job):
                    for s in subs:
                        p3jobs.append((fn, s))

            for p in range(NP):
                if p >= AVLAG:
                    q = p - AVLAG
                    av_emit(q)
                    if q % (HPC * PPH) == HPC * PPH - 1:
                        queue_ib(q // (HPC * PPH))
                sc_emit(p)
                if p3jobs and p % 3 == 2:
                    fn, s = p3jobs.popleft()
                    fn(s)
            for q in range(NP - AVLAG, NP):
                av_emit(q)
            queue_ib(IB - 1)
            while p3jobs:
                fn, s = p3jobs.popleft()
                fn(s)

    nc.finalize()
    return nc


def _get_nc():
    if "nc" not in _cache:
        _cache["nc"] = _build_nc()
    return _cache["nc"]


def kernel(x, Wq, Wk, Wv, Wo, bo):
    global last_exec_time_ns
    x = np.asarray(x, dtype=np.float32)
    Wq = np.asarray(Wq, dtype=np.float32)
    Wk = np.asarray(Wk, dtype=np.float32)
    Wv = np.asarray(Wv, dtype=np.float32)
    Wo = np.asarray(Wo, dtype=np.float32)
    bo = np.asarray(bo, dtype=np.float32)

    trace = bool(os.environ.get("BASS_KERNEL_TRACE"))
    if trace:
        _install_ntff_hook()
        import concourse.bass_utils as bass_utils
        bass_utils.upload_artifacts = lambda tmpdir: tmpdir

    nc = _get_nc()
    in_maps = []
    for c in range(NCORES):
        bi, hg = divmod(c, 2)
        s = slice(hg * FPC, (hg + 1) * FPC)
        in_maps.append({
            "xT": np.ascontiguousarray(x[bi].T).astype(BF16),
            "wq": np.ascontiguousarray(Wq[:, s]).astype(BF16),
            "wk": np.ascontiguousarray(Wk[:, s]).astype(BF16),
            "wv": np.ascontiguousarray(Wv[:, s]).astype(BF16),
            "wo": np.ascontiguousarray(Wo[s, :]).astype(BF16),
        })

    from concourse.bass_utils import run_bass_kernel_spmd
    res = run_bass_kernel_spmd(nc, in_maps, list(range(NCORES)), trace=trace)
    last_exec_time_ns = res.exec_time_ns

    parts = [res.results[c]["out"] for c in range(NCORES)]
    full = np.empty((B, N, DIM), np.float32)
    for bi in range(B):
        full[bi] = (parts[2 * bi].astype(np.float32)
                    + parts[2 * bi + 1].astype(np.float32) + bo[None, :])
    return full


# revision 9
# speedup vs baseline: 1.0713x; 1.0013x over previous
"""Multi-head attention (b=4, n=2048, dim=768, 12 heads) on 8 TRN2 NeuronCores.

Sharding: core c handles batch c//2 and head-group c%2 (6 of 12 heads).  Each
core computes its heads' contribution projected through its slice of Wo and
returns a partial [2048, 768] bf16 output; the host sums core pairs in f32 and
adds the bias.  No on-device collectives needed.

Per-core kernel (TensorE-facing data in bf16, accumulation in f32):
  P1: KT/QT = W^T x^T feature-major (x^T supplied pre-transposed by the host),
      V token-major in 128-wide head blocks whose column 64 is constant 1.
      Inputs land over two DMA queues (sync: x half 0; scalar: the rest); the
      V projection runs k-major over 8 PSUM tiles so the PE stream never
      blocks behind a chunk that has not landed, and K/Q are interleaved per
      feature tile so the first attention packs unblock as early as possible.
  P2: scores computed TRANSPOSED: ST[j,i] = K Q^T on PE (K=64 contraction),
      exp on ACT with scale 1/8 (softmax max-subtraction is unnecessary at
      these logit magnitudes), then OP[., i] += V'^T exp(ST) accumulated over
      j in PSUM.  The ones column of V' makes row 64 of OP the softmax
      denominator l[i] for free.  Unit order is (i-block, head, j-chunk) with
      4 j-chunks per exp tile; the attV matmuls trail the score matmuls by
      two packs so the PE never waits on ACT.
  P3: per 128 tokens: PE-transpose OP (bf16) to token-major, multiply by 1/l
      (per-partition scalar), PE-transpose back, project through Wo with
      PSUM accumulation.  P3 jobs for i-block k are interleaved between the
      P2 packs of i-block k+1, so there is no serial tail phase and output
      DMAs (bf16) spread across the whole run.
"""
import os
import sys
import types
from collections import deque
import numpy as np
import ml_dtypes

B, N, DIM = 4, 2048, 768
HEADS, DH = 12, 64
HPC = 6                # heads per core
FPC = HPC * DH         # 384 features per core
NCORES = 8
KC = DIM // 128        # 6 contraction chunks
FT = FPC // 128        # 3 feature tiles per core
NT = N // 128          # 16 token chunks of 128
IBS = 256              # i-block size
IB = N // IBS          # 8 i-blocks
JPP = 4                # j-chunks per pack (exp tile = [128, JPP*IBS])
PPH = NT // JPP        # 4 packs per (ib, h)
AVLAG = 2              # attV trails scores by this many packs
BF16 = ml_dtypes.bfloat16

_cache = {}
last_exec_time_ns = None


def _install_ntff_hook():
    try:
        import antenv.axon_hooks  # noqa: F401
        return
    except ImportError:
        pass
    from trn_agent_boot.trn_boot import _ntff_profile_via_ctypes
    hook = _ntff_profile_via_ctypes('/opt/axon/libaxon_pjrt.so')
    mod = types.ModuleType('antenv.axon_hooks')
    mod.get_axon_ntff_profile_hook = lambda: hook
    import antenv
    sys.modules['antenv.axon_hooks'] = mod
    antenv.axon_hooks = mod


def _build_nc():
    from contextlib import ExitStack
    from concourse import bacc
    import concourse.mybir as mybir
    from concourse.tile import TileContext
    from concourse.masks import make_identity
    from concourse.bass import broadcast_tensor_aps

    dt = mybir.dt
    EXP = mybir.ActivationFunctionType.Exp

    nc = bacc.Bacc("TRN2", target_bir_lowering=False, debug=False,
                   num_devices=NCORES)
    xT = nc.dram_tensor("xT", [DIM, N], dt.bfloat16, kind="ExternalInput").ap()
    wq = nc.dram_tensor("wq", [DIM, FPC], dt.bfloat16, kind="ExternalInput").ap()
    wk = nc.dram_tensor("wk", [DIM, FPC], dt.bfloat16, kind="ExternalInput").ap()
    wv = nc.dram_tensor("wv", [DIM, FPC], dt.bfloat16, kind="ExternalInput").ap()
    wo = nc.dram_tensor("wo", [FPC, DIM], dt.bfloat16, kind="ExternalInput").ap()
    out = nc.dram_tensor("out", [N, DIM], dt.bfloat16, kind="ExternalOutput").ap()

    with TileContext(nc) as tc, ExitStack() as ctx:
        const = ctx.enter_context(tc.tile_pool(name="const", bufs=1))
        id_bf = const.tile([128, 128], dt.bfloat16, tag="idb")
        make_identity(nc, id_bf)

        inp = ctx.enter_context(tc.tile_pool(name="inp", bufs=1))
        xts2 = [[inp.tile([128, N // 2], dt.bfloat16, tag=f"xt{k}_{hf}",
                          name=f"xt{k}_{hf}") for hf in range(2)]
                for k in range(KC)]
        wqs = [inp.tile([128, FPC], dt.bfloat16, tag=f"wq{k}", name=f"wq{k}")
               for k in range(KC)]
        wks = [inp.tile([128, FPC], dt.bfloat16, tag=f"wk{k}", name=f"wk{k}")
               for k in range(KC)]
        wvs = [inp.tile([128, FPC], dt.bfloat16, tag=f"wv{k}", name=f"wv{k}")
               for k in range(KC)]
        wos = [inp.tile([128, DIM], dt.bfloat16, tag=f"wo{f}", name=f"wo{f}")
               for f in range(FT)]
        # Two parallel DMA queues; ordering keeps the V projection fed from
        # ~1us (wv, x half 0) and both x halves landed by ~6us.
        for k in range(KC):
            nc.sync.dma_start(out=xts2[k][0][:],
                              in_=xT[k * 128:(k + 1) * 128, 0:N // 2])
            nc.scalar.dma_start(out=wvs[k][:], in_=wv[k * 128:(k + 1) * 128, :])
        for k in range(KC):
            nc.scalar.dma_start(out=xts2[k][1][:],
                               in_=xT[k * 128:(k + 1) * 128, N // 2:N])
        for k in range(KC):
            nc.scalar.dma_start(out=wks[k][:], in_=wk[k * 128:(k + 1) * 128, :])
        for k in range(KC):
            nc.scalar.dma_start(out=wqs[k][:], in_=wq[k * 128:(k + 1) * 128, :])
        for f in range(FT):
            nc.scalar.dma_start(out=wos[f][:], in_=wo[f * 128:(f + 1) * 128, :])

        kqv = ctx.enter_context(tc.tile_pool(name="kqv", bufs=1))
        KT = [kqv.tile([128, N], dt.bfloat16, tag=f"kt{f}", name=f"kt{f}")
              for f in range(FT)]
        QT = [kqv.tile([128, N], dt.bfloat16, tag=f"qt{f}", name=f"qt{f}")
              for f in range(FT)]
        VP = [kqv.tile([128, HPC * 128], dt.bfloat16, tag=f"vp{t}", name=f"vp{t}")
              for t in range(NT)]
        opsb = ctx.enter_context(tc.tile_pool(name="opsb", bufs=1))
        OPS = [[opsb.tile([65, IBS], dt.bfloat16, tag=f"op{h}_{ib}",
                          name=f"op{h}_{ib}") for ib in range(IB)]
               for h in range(HPC)]

        # ---- P1: projections ----
        # Zero VP fully first: columns 65-127 of each head block are never
        # written otherwise, and garbage there (potential Inf/NaN patterns)
        # would flow through the attV matmuls' unused output rows.
        for t in range(NT):
            nc.vector.memset(VP[t][:], 0.0)
            nc.vector.memset(
                VP[t].rearrange("p (h c) -> p h c", c=128)[:, :, 64:65], 1.0)
        with tc.tile_pool(name="p1ps", bufs=8, space="PSUM") as p1:
            # V projection k-major over 8 tiles at a time so the PE stream
            # never blocks behind a matmul whose x chunk has not landed yet.
            for half in range(2):
                pss = [p1.tile([128, FPC], dt.float32, tag="p1",
                               name=f"vps{half}_{i}") for i in range(8)]
                for k in range(KC):
                    for i in range(8):
                        nc.tensor.matmul(
                            pss[i][:],
                            lhsT=xts2[k][half][:, i * 128:(i + 1) * 128],
                            rhs=wvs[k][:], start=(k == 0), stop=(k == KC - 1))
                for i in range(8):
                    nc.vector.tensor_copy(
                        VP[half * 8 + i].rearrange(
                            "p (h c) -> p h c", c=128)[:, :, 0:64],
                        pss[i].rearrange("p (h c) -> p h c", c=64))
            # K and Q interleaved per feature tile so the first P2 packs
            # (which need KT[0] and QT[0]) are unblocked as early as possible.
            for f in range(FT):
                for W, DST in ((wks, KT), (wqs, QT)):
                    for q in range(N // 512):
                        ps = p1.tile([128, 512], dt.float32, tag="p1",
                                     name=f"kqps{f}_{q}")
                        for k in range(KC):
                            nc.tensor.matmul(
                                ps[:], lhsT=W[k][:, f * 128:(f + 1) * 128],
                                rhs=xts2[k][q // 2][:, (q % 2) * 512:
                                                    (q % 2 + 1) * 512],
                                start=(k == 0), stop=(k == KC - 1))
                        nc.vector.tensor_copy(DST[f][:, q * 512:(q + 1) * 512],
                                              ps[:])

        # ---- P2 + P3 fused ----
        packs = [(ib, h, g) for ib in range(IB) for h in range(HPC)
                 for g in range(PPH)]
        NP = len(packs)

        with tc.tile_pool(name="p2st", bufs=2, space="PSUM") as p2st, \
                tc.tile_pool(name="p2op", bufs=2, space="PSUM") as p2op, \
                tc.tile_pool(name="p3ps", bufs=2, space="PSUM") as p3ps, \
                tc.tile_pool(name="expp", bufs=3) as expp, \
                tc.tile_pool(name="otokp", bufs=2) as otokp, \
                tc.tile_pool(name="otnp", bufs=2) as otnp, \
                tc.tile_pool(name="linvp", bufs=2) as linvp, \
                tc.tile_pool(name="outst", bufs=2) as outst:
            ops = {}
            exs = {}

            def sc_emit(p):
                ib, h, g = packs[p]
                ktf, qtf, r0 = KT[h // 2], QT[h // 2], (h % 2) * 64
                st = p2st.tile([128, JPP * IBS], dt.float32, tag="st",
                               name=f"st{p}")
                for u in range(JPP):
                    jc = g * JPP + u
                    nc.tensor.matmul(
                        st[:, u * IBS:(u + 1) * IBS],
                        lhsT=ktf[r0:r0 + 64, jc * 128:(jc + 1) * 128],
                        rhs=qtf[r0:r0 + 64, ib * IBS:(ib + 1) * IBS],
                        start=True, stop=True)
                ex = expp.tile([128, JPP * IBS], dt.bfloat16, tag="ex",
                               name=f"ex{p}")
                nc.scalar.activation(ex[:], st[:], EXP, scale=0.125)
                exs[p] = ex

            def av_emit(p):
                ib, h, g = packs[p]
                ex = exs.pop(p)
                if g == 0:
                    ops[(h, ib)] = p2op.tile([128, IBS], dt.float32,
                                             tag="op", name=f"opp{h}_{ib}")
                op = ops[(h, ib)]
                for u in range(JPP):
                    jc = g * JPP + u
                    nc.tensor.matmul(
                        op[:], lhsT=VP[jc][:, h * 128:(h + 1) * 128],
                        rhs=ex[:, u * IBS:(u + 1) * IBS],
                        start=(jc == 0), stop=(jc == NT - 1))
                if g == PPH - 1:
                    op = ops.pop((h, ib))
                    nc.vector.tensor_copy(OPS[h][ib][:], op[0:65, :])

            otoks = {}
            otns = {}

            def tr_job(isub):
                ib, col = isub // (IBS // 128), (isub % (IBS // 128)) * 128
                # 66-wide per-head slots keep bf16 PSUM writes 4B-aligned
                trp = p3ps.tile([128, HPC * 66], dt.bfloat16, tag="p3",
                                name=f"trp{isub}")
                for h in range(HPC):
                    nc.tensor.transpose(trp[:, h * 66:h * 66 + 65],
                                        OPS[h][ib][0:65, col:col + 128],
                                        id_bf[0:65, 0:65])
                trv = trp.rearrange("p (h c) -> p h c", c=66)
                linv6 = linvp.tile([128, HPC, 1], dt.float32, tag="l6",
                                   name=f"l6{isub}")
                nc.vector.reciprocal(linv6[:], trv[:, :, 64:65])
                otok = otokp.tile([128, FPC], dt.bfloat16, tag="otok",
                                  name=f"otok{isub}")
                a, b = broadcast_tensor_aps(trv[:, :, 0:64], linv6[:])
                nc.vector.tensor_mul(
                    otok.rearrange("p (h c) -> p h c", c=64), a, b)
                otoks[isub] = otok

            def tb_job(isub):
                otok = otoks.pop(isub)
                tbp = p3ps.tile([128, FPC], dt.bfloat16, tag="p3",
                                name=f"tbp{isub}")
                for f in range(FT):
                    nc.tensor.transpose(tbp[:, f * 128:(f + 1) * 128],
                                        otok[:, f * 128:(f + 1) * 128],
                                        id_bf[:])
                otn = otnp.tile([128, FPC], dt.bfloat16, tag="otn",
                                name=f"otn{isub}")
                nc.vector.tensor_copy(otn[:], tbp[:])
                otns[isub] = otn

            def proj_job(isub):
                otn = otns.pop(isub)
                ob = outst.tile([128, DIM], dt.bfloat16, tag="ob",
                                name=f"ob{isub}")
                for half in range(2):
                    pp = p3ps.tile([128, DIM // 2], dt.float32, tag="p3",
                                   name=f"pp{isub}_{half}")
                    for f in range(FT):
                        nc.tensor.matmul(
                            pp[:], lhsT=otn[:, f * 128:(f + 1) * 128],
                            rhs=wos[f][:, half * 384:(half + 1) * 384],
                            start=(f == 0), stop=(f == FT - 1))
                    nc.vector.tensor_copy(ob[:, half * 384:(half + 1) * 384],
                                          pp[:])
                nc.sync.dma_start(out=out[isub * 128:(isub + 1) * 128, :],
                                  in_=ob[:])

            p3jobs = deque()

            def queue_ib(ib):
                subs = range(ib * (IBS // 128), (ib + 1) * (IBS // 128))
                for fn in (tr_job, tb_job, proj_job):
                    for s in subs:
                        p3jobs.append((fn, s))

            for p in range(NP):
                if p >= AVLAG:
                    q = p - AVLAG
                    av_emit(q)
                    if q % (HPC * PPH) == HPC * PPH - 1:
                        queue_ib(q // (HPC * PPH))
                sc_emit(p)
                if p3jobs and p % 3 == 2:
                    fn, s = p3jobs.popleft()
                    fn(s)
            for q in range(NP - AVLAG, NP):
                av_emit(q)
            queue_ib(IB - 1)
            while p3jobs:
                fn, s = p3jobs.popleft()
                fn(s)

    nc.finalize()
    return nc


def _get_nc():
    if "nc" not in _cache:
        _cache["nc"] = _build_nc()
    return _cache["nc"]


def kernel(x, Wq, Wk, Wv, Wo, bo):
    global last_exec_time_ns
    x = np.asarray(x, dtype=np.float32)
    Wq = np.asarray(Wq, dtype=np.float32)
    Wk = np.asarray(Wk, dtype=np.float32)
    Wv = np.asarray(Wv, dtype=np.float32)
    Wo = np.asarray(Wo, dtype=np.float32)
    bo = np.asarray(bo, dtype=np.float32)

    trace = bool(os.environ.get("BASS_KERNEL_TRACE"))
    if trace:
        _install_ntff_hook()
        import concourse.bass_utils as bass_utils
        bass_utils.upload_artifacts = lambda tmpdir: tmpdir

    nc = _get_nc()
    in_maps = []
    for c in range(NCORES):
        bi, hg = divmod(c, 2)
        s = slice(hg * FPC, (hg + 1) * FPC)
        in_maps.append({
            "xT": np.ascontiguousarray(x[bi].T).astype(BF16),
            "wq": np.ascontiguousarray(Wq[:, s]).astype(BF16),
            "wk": np.ascontiguousarray(Wk[:, s]).astype(BF16),
            "wv": np.ascontiguousarray(Wv[:, s]).astype(BF16),
            "wo": np.ascontiguousarray(Wo[s, :]).astype(BF16),
        })

    from concourse.bass_utils import run_bass_kernel_spmd
    res = run_bass_kernel_spmd(nc, in_maps, list(range(NCORES)), trace=trace)
    last_exec_time_ns = res.exec_time_ns

    parts = [res.results[c]["out"] for c in range(NCORES)]
    full = np.empty((B, N, DIM), np.float32)
    for bi in range(B):
        full[bi] = (parts[2 * bi].astype(np.float32)
                    + parts[2 * bi + 1].astype(np.float32) + bo[None, :])
    return full
